# revision 14
# baseline (speedup 1.0000x reference)
"""Expert-parallel MoE (top-2 of 8 experts, SwiGLU FFN) for 8 Trainium2 cores.

Strategy (expert-parallel per the sharding hint, plus load balancing):
  - Host computes the small gate (logits -> top-2 -> softmax) in float64
    numpy, then dispatches tokens to experts.
  - Device work is the 3-matmul SwiGLU FFN in bf16 (1 PE cycle/row, FWL
    hides LDWEIGHTS; rel err ~4e-3 vs the 2e-2 gate). fp8-DoubleRow was
    measured unusable: e4m3 on any one matmul already gives >3.7e-2.
  - Load balancing: per-expert token counts are imbalanced (max 2151 vs
    mean 2048).  Instead of padding every core to the max count, each
    core runs THREE fixed-size slots (a, b, c); the biggest expert's
    tokens span two a-slots (on different cores), the smallest expert
    donates capacity by using two b-slots, and everyone gets one
    c-slot.  This brings per-core capacity S = a+b+c down to
    max(d2, ceil((d1+d8)/2)) ~ 2068 instead of 2152.  Each slot streams
    its own expert's weights (3 x 17.3MB bf16 per core, well under the
    HBM roofline vs ~455us of compute).
  - The device kernel works in "feature-major" layout (features on
    partitions, tokens on the free axis) so h = silu(x@Wg)*(x@Wu)
    feeds the down-projection without any transpose.
  - Host applies routing weights and scatter-adds per-slot outputs
    back into the full [B,T,D] output.
"""

import numpy as np

DIM = 1024
HID = 2816
E = 8
TOPK = 2
P = 128
KD = DIM // P   # 8 k-subtiles (contraction of x@W)
HT = HID // P   # 22 h-subtiles
DT = DIM // P   # 8 d-subtiles (output features)

CHUNK_MAX = 512         # PSUM bank limit (512 fp32 accumulators)

_KERNEL_CACHE = {}
LAST_RESULTS = None  # BassKernelResults of the most recent run (for profiling)


def _ceil_even(v):
    return ((int(v) + 1) // 2) * 2


def _split_chunks(size):
    """Split a slot into even chunks <= CHUNK_MAX (one PSUM bank each),
    near-equal so every chunk's matmuls stay long enough to hide
    LDWEIGHTS."""
    n = -(-size // CHUNK_MAX)
    sizes = []
    rem = size
    for i in range(n, 0, -1):
        s = rem if i == 1 else min(CHUNK_MAX, ((rem // i + 1) // 2) * 2)
        sizes.append(s)
        rem -= s
    assert rem == 0 and all(0 < s <= CHUNK_MAX and s % 2 == 0 for s in sizes)
    chunks = []
    off = 0
    for s in sizes:
        chunks.append((off, s))
        off += s
    return chunks


def _plan(counts):
    """Choose per-core slot sizes + (expert, lo, hi) piece assignment.

    Slot types (a, b, c), 8 copies of each (one per core):
      biggest expert  -> {a, a, c}   (its surplus spans two cores' a-slots)
      smallest expert -> {b, b, c}   (donates capacity: b < a)
      middle experts  -> {a, b, c}
    Coverage: 2a+c >= d_max, 2b+c >= d_min, a+b+c >= d_2nd, so
    S = a+b+c = max(d_2nd, ceil((d_max+d_min)/2)) is achievable.
    Returns (sizes, assign) with sizes desc-sorted and
    assign[core] = [(expert, lo, hi), ...] parallel to sizes.
    """
    cnts = [int(x) for x in counts]
    order = sorted(range(E), key=lambda e: -cnts[e])
    d = [cnts[e] for e in order]
    d0, d1, dmin = d[0], d[1], d[-1]

    smin = _ceil_even(max(d1, (d0 + dmin + 1) // 2, 6))
    best = None
    for c in range(384, 1153, 32):
        if c >= smin - 4:
            break
        S = smin
        plan_ab = None
        while S <= _ceil_even(d0):
            a = _ceil_even(-(-max(d0 - c, 2) // 2))
            b = S - a - c
            if b >= 2 and 2 * b + c >= dmin:
                plan_ab = (a, b)
                break
            S += 2
        if plan_ab is None:
            continue
        a, b = plan_ab
        minchunk = min(
            min(cs for _, cs in _split_chunks(s)) for s in (a, b, c)
        )
        key = (S, -minchunk)
        if best is None or key < best[0]:
            best = (key, (a, b, c))

    if best is None:
        # fallback: uniform single slot per core (always correct)
        C = _ceil_even(max(max(cnts), 256))
        sizes = [C]
        assign = [[(order[i], 0, cnts[order[i]])] for i in range(E)]
        return sizes, assign

    a, b, c = best[1]

    def pieces(n, caps):
        out = []
        lo = 0
        for cap in caps:
            hi = min(n, lo + cap)
            out.append((lo, hi))
            lo = hi
        assert lo == n
        return out

    a_pieces, b_pieces, c_pieces = [], [], []
    for rank, e in enumerate(order):
        n = cnts[e]
        if rank == 0:
            pa1, pa2, pc = pieces(n, [a, a, c])
            a_pieces += [(e,) + pa1, (e,) + pa2]
            c_pieces.append((e,) + pc)
        elif rank == E - 1:
            pb1, pb2, pc = pieces(n, [b, b, c])
            b_pieces += [(e,) + pb1, (e,) + pb2]
            c_pieces.append((e,) + pc)
        else:
            pa, pb, pc = pieces(n, [a, b, c])
            a_pieces.append((e,) + pa)
            b_pieces.append((e,) + pb)
            c_pieces.append((e,) + pc)
    assert len(a_pieces) == E and len(b_pieces) == E and len(c_pieces) == E

    typed = sorted(
        [(a, a_pieces), (b, b_pieces), (c, c_pieces)], key=lambda t: -t[0]
    )
    sizes = [t[0] for t in typed]
    assign = [[typed[s][1][i] for s in range(len(typed))] for i in range(E)]
    return sizes, assign


def _build_moe_ffn(slot_sizes):
    """Per-core Bass program: for each slot, y^T = SwiGLU FFN of x^T with
    that slot's own expert weights; feature-major, bf16 matmuls."""
    import concourse.bass as bass  # noqa: F401
    import concourse.mybir as mybir
    from concourse import bacc, tile

    f32 = mybir.dt.float32
    dt_in = mybir.dt.bfloat16
    SiLU = mybir.ActivationFunctionType.Silu

    nc = bacc.Bacc("TRN2", target_bir_lowering=False, debug=False)

    xt, wgt, wut, wdt, yt = [], [], [], [], []
    for s, size in enumerate(slot_sizes):
        xt.append(nc.dram_tensor(f"xt{s}", [P, KD * size], dt_in,
                                 kind="ExternalInput"))
        wgt.append(nc.dram_tensor(f"wgt{s}", [HT, P, KD, P], dt_in,
                                  kind="ExternalInput"))
        wut.append(nc.dram_tensor(f"wut{s}", [HT, P, KD, P], dt_in,
                                  kind="ExternalInput"))
        wdt.append(nc.dram_tensor(f"wdt{s}", [DT, P, HT, P], dt_in,
                                  kind="ExternalInput"))
        yt.append(nc.dram_tensor(f"yt{s}", [DT, P, size], dt_in,
                                 kind="ExternalOutput"))

    with tile.TileContext(nc) as tc:
        with (
            tc.tile_pool(name="xp", bufs=1) as xp,
            tc.tile_pool(name="wp", bufs=3) as wp,
            tc.tile_pool(name="hp", bufs=2) as hp,
            tc.tile_pool(name="op", bufs=3) as op,
            tc.tile_pool(name="ps", bufs=2, space="PSUM") as ps,
        ):
            HH = KD // 2  # wg/wu tiles split in halves for earlier start

            # Warm-up: ~10us of dummy matmuls on memset tiles keep the
            # PE HAM clock-gate busy during the opening DMA fill, so the
            # first real matmuls run at 2.4GHz instead of 1.2.
            wdum = op.tile([P, P], mybir.dt.bfloat16, tag="wdum", bufs=1)
            nc.vector.memset(wdum[:], 0.0)
            xdum = op.tile([P, CHUNK_MAX], mybir.dt.bfloat16, tag="xdum",
                           bufs=1)
            nc.vector.memset(xdum[:], 0.0)
            for _ in range(12):
                pwarm = ps.tile([P, CHUNK_MAX], mybir.dt.float32,
                                tag="pg", bufs=3, name="pwarm")
                nc.tensor.matmul(pwarm, wdum[:], xdum[:],
                                 start=True, stop=True)

            for s, size in enumerate(slot_sizes):
                group = _split_chunks(size)

                # h = silu(x @ Wg) * (x @ Wu), feature-major [HID, size]
                h_sb = hp.tile([P, HT, size], dt_in, tag="h")

                w_cache = {}

                def load_wg(ht, hh, s=s):
                    w1 = wp.tile([P, HH, P], dt_in, tag=f"wg{hh}",
                                 name=f"wg{hh}")
                    nc.sync.dma_start(
                        w1[:], wgt[s][ht, :, hh * HH : (hh + 1) * HH])
                    return w1

                def load_wu(ht, hh, s=s):
                    w2 = wp.tile([P, HH, P], dt_in, tag=f"wu{hh}",
                                 name=f"wu{hh}")
                    nc.sync.dma_start(
                        w2[:], wut[s][ht, :, hh * HH : (hh + 1) * HH])
                    return w2

                def load_w(ht):
                    # both wg halves before wu: the opening matmuls of
                    # each h-tile consume wg only
                    return ([load_wg(ht, 0), load_wg(ht, 1)],
                            [load_wu(ht, 0), load_wu(ht, 1)])

                # Opening order interleaves the first chunk's x k-slices
                # with ht=0's weight tiles so the first matmul waits on
                # ~240KB and each successive k-group lands just in time.
                x_sb = [None] * len(group)

                def load_x(gi, off, csize, s=s):
                    # chunk-contiguous x: k0 as its own small DMA so the
                    # opening matmul waits on ~110KB; k1..7 in ONE DMA
                    # with ~6KB-per-partition descriptors
                    base = KD * off
                    x0 = xp.tile([P, csize], dt_in, tag=f"x{gi}k0",
                                 bufs=2 if gi == 0 else 1, name=f"x{gi}k0")
                    nc.sync.dma_start(
                        x0[:], xt[s][:, base : base + csize])
                    wg0 = load_wg(0, 0) if gi == 0 and s == 0 else None
                    xr = xp.tile([P, (KD - 1) * csize], dt_in, tag=f"x{gi}r",
                                 bufs=2 if gi == 0 else 1, name=f"x{gi}r")
                    nc.sync.dma_start(
                        xr[:], xt[s][:, base + csize : base + KD * csize])
                    wg1 = load_wg(0, 1) if gi == 0 and s == 0 else None
                    x_sb[gi] = ([x0] + [
                        xr[:, kt * csize : (kt + 1) * csize]
                        for kt in range(KD - 1)])
                    return (wg0, wg1)

                if s == 0:
                    off0, csize0 = group[0]
                    wg0, wg1 = load_x(0, off0, csize0)
                    w_cache[0] = ([wg0, wg1],
                                  [load_wu(0, 0), load_wu(0, 1)])
                    w_cache[1] = load_w(1)
                    for gi, (off, csize) in enumerate(group):
                        if gi:
                            load_x(gi, off, csize)
                else:
                    w_cache[0] = load_w(0)
                    w_cache[1] = load_w(1)
                    for gi, (off, csize) in enumerate(group):
                        load_x(gi, off, csize)

                for ht in range(HT):
                    if ht not in w_cache:
                        w_cache[ht] = load_w(ht)
                    wg_sb, wu_sb = w_cache.pop(ht)

                    for gi, (off, csize) in enumerate(group):
                        pg = ps.tile([P, csize], f32, tag="pg", bufs=3)
                        pu = ps.tile([P, csize], f32, tag="pu", bufs=3)
                        for kt in range(KD):
                            nc.tensor.matmul(
                                pg,
                                wg_sb[kt // HH][:, kt % HH],
                                x_sb[gi][kt],
                                start=(kt == 0),
                                stop=(kt == KD - 1),
                            )
                        for kt in range(KD):
                            nc.tensor.matmul(
                                pu,
                                wu_sb[kt // HH][:, kt % HH],
                                x_sb[gi][kt],
                                start=(kt == 0),
                                stop=(kt == KD - 1),
                            )
                        sl = op.tile([P, csize], f32, tag="silu")
                        nc.scalar.activation(sl[:], pg, SiLU)
                        nc.vector.tensor_mul(
                            h_sb[:, ht, off : off + csize], sl[:], pu
                        )

                # y = h @ Wd, feature-major [DIM, size]
                for dt in range(DT):
                    wd_sb = wp.tile([P, HT, P], dt_in, tag="wd")
                    nc.sync.dma_start(wd_sb[:], wdt[s][dt])
                    for gi, (off, csize) in enumerate(group):
                        py = ps.tile([P, csize], f32, tag="py")
                        for ht in range(HT):
                            nc.tensor.matmul(
                                py,
                                wd_sb[:, ht],
                                h_sb[:, ht, off : off + csize],
                                start=(ht == 0),
                                stop=(ht == HT - 1),
                            )
                        o_sb = op.tile([P, csize], dt_in, tag="o")
                        nc.vector.tensor_copy(o_sb[:], py)
                        nc.sync.dma_start(
                            yt[s][dt, :, off : off + csize], o_sb[:])

    nc.finalize()
    return nc


def _get_kernel(slot_sizes):
    key = tuple(slot_sizes)
    if key not in _KERNEL_CACHE:
        _KERNEL_CACHE[key] = _build_moe_ffn(list(slot_sizes))
    return _KERNEL_CACHE[key]


def _np_bf16():
    import ml_dtypes

    return np.dtype(ml_dtypes.bfloat16)


def _route(xf, W_gate):
    """Replicate reference routing: top-2 by logit, softmax weights.

    float64 logits: the top-k decision boundary gap is >> f32 rounding
    noise, so this matches the f32 jax reference's selection."""
    logits = xf.astype(np.float64) @ W_gate.astype(np.float64)  # [N, E]
    order = np.argsort(-logits, axis=1, kind="stable")[:, :TOPK]  # [N, 2]
    top = np.take_along_axis(logits, order, axis=1)
    top = top - top.max(axis=1, keepdims=True)
    ew = np.exp(top)
    w = (ew / ew.sum(axis=1, keepdims=True)).astype(np.float32)  # [N, 2]
    return order, w


def kernel(x, W_gate, Wg, Wu, Wd):
    from concourse.bass_utils import run_bass_kernel_spmd

    x = np.ascontiguousarray(np.asarray(x, dtype=np.float32))
    W_gate = np.asarray(W_gate, dtype=np.float32)
    Wg = np.asarray(Wg, dtype=np.float32)
    Wu = np.asarray(Wu, dtype=np.float32)
    Wd = np.asarray(Wd, dtype=np.float32)

    B, T, D = x.shape
    xf = x.reshape(-1, D)
    N = xf.shape[0]

    order, w = _route(xf, W_gate)

    ids = []  # per-expert token indices
    wts = []  # per-expert combine weights
    for e in range(E):
        sel = np.nonzero(order == e)
        ids.append(sel[0])
        wts.append(w[sel[0], sel[1]])

    sizes, assign = _plan([len(i) for i in ids])
    nc = _get_kernel(sizes)
    ndt = _np_bf16()

    # transpose + bf16-cast each expert's weights once
    wcache = {}

    def get_w(e):
        if e not in wcache:
            wg_t = np.ascontiguousarray(
                Wg[e].reshape(KD, P, HT, P).transpose(2, 1, 0, 3)
                .astype(ndt, copy=False))
            wu_t = np.ascontiguousarray(
                Wu[e].reshape(KD, P, HT, P).transpose(2, 1, 0, 3)
                .astype(ndt, copy=False))
            wd_t = np.ascontiguousarray(
                Wd[e].reshape(HT, P, DT, P).transpose(2, 1, 0, 3)
                .astype(ndt, copy=False))
            wcache[e] = (wg_t, wu_t, wd_t)
        return wcache[e]

    in_maps = []
    for core in range(E):
        im = {}
        for s, size in enumerate(sizes):
            e, lo, hi = assign[core][s]
            cnt = hi - lo
            xe = np.zeros((size, DIM), dtype=np.float32)
            if cnt:
                xe[:cnt] = xf[ids[e][lo:hi]]
            xfm = xe.T.reshape(KD, P, size)
            parts = [
                xfm[:, :, off : off + cs].transpose(1, 0, 2)
                .reshape(P, KD * cs)
                for off, cs in _split_chunks(size)
            ]
            im[f"xt{s}"] = np.ascontiguousarray(
                np.concatenate(parts, axis=1).astype(ndt, copy=False))
            wg_t, wu_t, wd_t = get_w(e)
            im[f"wgt{s}"] = wg_t
            im[f"wut{s}"] = wu_t
            im[f"wdt{s}"] = wd_t
        in_maps.append(im)

    res = run_bass_kernel_spmd(nc, in_maps, core_ids=list(range(E)))
    global LAST_RESULTS
    LAST_RESULTS = res

    out = np.zeros((N, D), dtype=np.float32)
    for core in range(E):
        for s, size in enumerate(sizes):
            e, lo, hi = assign[core][s]
            cnt = hi - lo
            if not cnt:
                continue
            y_e = (res.results[core][f"yt{s}"].astype(np.float32)
                   .reshape(DIM, size)[:, :cnt].T)
            out[ids[e][lo:hi]] += wts[e][lo:hi, None] * y_e
    return out.reshape(B, T, D)


# revision 17
# speedup vs baseline: 1.0012x; 1.0012x over previous
"""Expert-parallel MoE (top-2 of 8 experts, SwiGLU FFN) for 8 Trainium2 cores.

Strategy (expert-parallel per the sharding hint, plus load balancing):
  - Host computes the small gate (logits -> top-2 -> softmax) in float64
    numpy, then dispatches tokens to experts.
  - Device work is the 3-matmul SwiGLU FFN in bf16 (1 PE cycle/row, FWL
    hides LDWEIGHTS; rel err ~4e-3 vs the 2e-2 gate). fp8-DoubleRow was
    measured unusable: e4m3 on any one matmul already gives >3.7e-2.
  - Load balancing: per-expert token counts are imbalanced (max 2151 vs
    mean 2048).  Instead of padding every core to the max count, each
    core runs THREE fixed-size slots (a, b, c); the biggest expert's
    tokens span two a-slots (on different cores), the smallest expert
    donates capacity by using two b-slots, and everyone gets one
    c-slot.  This brings per-core capacity S = a+b+c down to
    max(d2, ceil((d1+d8)/2)) ~ 2068 instead of 2152.  Each slot streams
    its own expert's weights (3 x 17.3MB bf16 per core, well under the
    HBM roofline vs ~455us of compute).
  - The device kernel works in "feature-major" layout (features on
    partitions, tokens on the free axis) so h = silu(x@Wg)*(x@Wu)
    feeds the down-projection without any transpose.
  - Host applies routing weights and scatter-adds per-slot outputs
    back into the full [B,T,D] output.
"""

import numpy as np

DIM = 1024
HID = 2816
E = 8
TOPK = 2
P = 128
KD = DIM // P   # 8 k-subtiles (contraction of x@W)
HT = HID // P   # 22 h-subtiles
DT = DIM // P   # 8 d-subtiles (output features)

CHUNK_MAX = 512         # PSUM bank limit (512 fp32 accumulators)

_KERNEL_CACHE = {}
LAST_RESULTS = None  # BassKernelResults of the most recent run (for profiling)


def _ceil_even(v):
    return ((int(v) + 1) // 2) * 2


def _split_chunks(size):
    """Split a slot into even chunks <= CHUNK_MAX (one PSUM bank each),
    near-equal so every chunk's matmuls stay long enough to hide
    LDWEIGHTS."""
    n = -(-size // CHUNK_MAX)
    sizes = []
    rem = size
    for i in range(n, 0, -1):
        s = rem if i == 1 else min(CHUNK_MAX, ((rem // i + 1) // 2) * 2)
        sizes.append(s)
        rem -= s
    assert rem == 0 and all(0 < s <= CHUNK_MAX and s % 2 == 0 for s in sizes)
    chunks = []
    off = 0
    for s in sizes:
        chunks.append((off, s))
        off += s
    return chunks


def _plan(counts):
    """Choose per-core slot sizes + (expert, lo, hi) piece assignment.

    Slot types (a, b, c), 8 copies of each (one per core):
      biggest expert  -> {a, a, c}   (its surplus spans two cores' a-slots)
      smallest expert -> {b, b, c}   (donates capacity: b < a)
      middle experts  -> {a, b, c}
    Coverage: 2a+c >= d_max, 2b+c >= d_min, a+b+c >= d_2nd, so
    S = a+b+c = max(d_2nd, ceil((d_max+d_min)/2)) is achievable.
    Returns (sizes, assign) with sizes desc-sorted and
    assign[core] = [(expert, lo, hi), ...] parallel to sizes.
    """
    cnts = [int(x) for x in counts]
    order = sorted(range(E), key=lambda e: -cnts[e])
    d = [cnts[e] for e in order]
    d0, d1, dmin = d[0], d[1], d[-1]

    smin = _ceil_even(max(d1, (d0 + dmin + 1) // 2, 6))
    best = None
    for c in range(384, 1153, 32):
        if c >= smin - 4:
            break
        S = smin
        plan_ab = None
        while S <= _ceil_even(d0):
            a = _ceil_even(-(-max(d0 - c, 2) // 2))
            b = S - a - c
            if b >= 2 and 2 * b + c >= dmin:
                plan_ab = (a, b)
                break
            S += 2
        if plan_ab is None:
            continue
        a, b = plan_ab
        minchunk = min(
            min(cs for _, cs in _split_chunks(s)) for s in (a, b, c)
        )
        key = (S, -minchunk)
        if best is None or key < best[0]:
            best = (key, (a, b, c))

    if best is None:
        # fallback: uniform single slot per core (always correct)
        C = _ceil_even(max(max(cnts), 256))
        sizes = [C]
        assign = [[(order[i], 0, cnts[order[i]])] for i in range(E)]
        return sizes, assign

    a, b, c = best[1]

    def pieces(n, caps):
        out = []
        lo = 0
        for cap in caps:
            hi = min(n, lo + cap)
            out.append((lo, hi))
            lo = hi
        assert lo == n
        return out

    a_pieces, b_pieces, c_pieces = [], [], []
    for rank, e in enumerate(order):
        n = cnts[e]
        if rank == 0:
            pa1, pa2, pc = pieces(n, [a, a, c])
            a_pieces += [(e,) + pa1, (e,) + pa2]
            c_pieces.append((e,) + pc)
        elif rank == E - 1:
            pb1, pb2, pc = pieces(n, [b, b, c])
            b_pieces += [(e,) + pb1, (e,) + pb2]
            c_pieces.append((e,) + pc)
        else:
            pa, pb, pc = pieces(n, [a, b, c])
            a_pieces.append((e,) + pa)
            b_pieces.append((e,) + pb)
            c_pieces.append((e,) + pc)
    assert len(a_pieces) == E and len(b_pieces) == E and len(c_pieces) == E

    typed = sorted(
        [(a, a_pieces), (b, b_pieces), (c, c_pieces)], key=lambda t: -t[0]
    )
    sizes = [t[0] for t in typed]
    assign = [[typed[s][1][i] for s in range(len(typed))] for i in range(E)]
    return sizes, assign


def _build_moe_ffn(slot_sizes):
    """Per-core Bass program: for each slot, y^T = SwiGLU FFN of x^T with
    that slot's own expert weights; feature-major, bf16 matmuls."""
    import concourse.bass as bass  # noqa: F401
    import concourse.mybir as mybir
    from concourse import bacc, tile

    f32 = mybir.dt.float32
    dt_in = mybir.dt.bfloat16
    SiLU = mybir.ActivationFunctionType.Silu

    nc = bacc.Bacc("TRN2", target_bir_lowering=False, debug=False)

    xt, wgt, wut, wdt, yt = [], [], [], [], []
    for s, size in enumerate(slot_sizes):
        xt.append(nc.dram_tensor(f"xt{s}", [P, KD, size], dt_in,
                                 kind="ExternalInput"))
        wgt.append(nc.dram_tensor(f"wgt{s}", [HT, P, KD, P], dt_in,
                                  kind="ExternalInput"))
        wut.append(nc.dram_tensor(f"wut{s}", [HT, P, KD, P], dt_in,
                                  kind="ExternalInput"))
        wdt.append(nc.dram_tensor(f"wdt{s}", [DT, P, HT, P], dt_in,
                                  kind="ExternalInput"))
        yt.append(nc.dram_tensor(f"yt{s}", [DT, P, size], dt_in,
                                 kind="ExternalOutput"))

    with tile.TileContext(nc) as tc:
        with (
            tc.tile_pool(name="xp", bufs=1) as xp,
            tc.tile_pool(name="wp", bufs=3) as wp,
            tc.tile_pool(name="hp", bufs=2) as hp,
            tc.tile_pool(name="op", bufs=3) as op,
            tc.tile_pool(name="ps", bufs=2, space="PSUM") as ps,
        ):
            HH = KD // 2  # wg/wu tiles split in halves for earlier start

            # Warm-up: ~10us of dummy matmuls on memset tiles keep the
            # PE HAM clock-gate busy during the opening DMA fill, so the
            # first real matmuls run at 2.4GHz instead of 1.2.
            wdum = op.tile([P, P], mybir.dt.bfloat16, tag="wdum", bufs=1)
            nc.vector.memset(wdum[:], 0.0)
            xdum = op.tile([P, CHUNK_MAX], mybir.dt.bfloat16, tag="xdum",
                           bufs=1)
            nc.vector.memset(xdum[:], 0.0)
            for _ in range(12):
                pwarm = ps.tile([P, CHUNK_MAX], mybir.dt.float32,
                                tag="pg", bufs=3, name="pwarm")
                nc.tensor.matmul(pwarm, wdum[:], xdum[:],
                                 start=True, stop=True)

            for s, size in enumerate(slot_sizes):
                group = _split_chunks(size)

                # h = silu(x @ Wg) * (x @ Wu), feature-major [HID, size]
                h_sb = hp.tile([P, HT, size], dt_in, tag="h")

                w_cache = {}

                def load_wg(ht, hh, s=s):
                    w1 = wp.tile([P, HH, P], dt_in, tag=f"wg{hh}",
                                 name=f"wg{hh}")
                    nc.sync.dma_start(
                        w1[:], wgt[s][ht, :, hh * HH : (hh + 1) * HH])
                    return w1

                def load_wu(ht, hh, s=s):
                    w2 = wp.tile([P, HH, P], dt_in, tag=f"wu{hh}",
                                 name=f"wu{hh}")
                    nc.sync.dma_start(
                        w2[:], wut[s][ht, :, hh * HH : (hh + 1) * HH])
                    return w2

                def load_w(ht):
                    # both wg halves before wu: the opening matmuls of
                    # each h-tile consume wg only
                    return ([load_wg(ht, 0), load_wg(ht, 1)],
                            [load_wu(ht, 0), load_wu(ht, 1)])

                # Opening order interleaves the first chunk's x k-slices
                # with ht=0's weight tiles so the first matmul waits on
                # ~240KB and each successive k-group lands just in time.
                x_sb = [None] * len(group)

                def load_x(gi, off, csize, s=s):
                    x0 = xp.tile([P, csize], dt_in, tag=f"x{gi}k0",
                                 bufs=2 if gi == 0 else 1, name=f"x{gi}k0")
                    nc.sync.dma_start(x0[:], xt[s][:, 0, off : off + csize])
                    wg0 = load_wg(0, 0) if gi == 0 and s == 0 else None
                    xa = xp.tile([P, HH - 1, csize], dt_in, tag=f"x{gi}a",
                                 bufs=2 if gi == 0 else 1, name=f"x{gi}a")
                    nc.sync.dma_start(xa[:], xt[s][:, 1:HH, off : off + csize])
                    wg1 = load_wg(0, 1) if gi == 0 and s == 0 else None
                    xb = xp.tile([P, KD - HH, csize], dt_in, tag=f"x{gi}b",
                                 bufs=2 if gi == 0 else 1, name=f"x{gi}b")
                    nc.sync.dma_start(xb[:], xt[s][:, HH:, off : off + csize])
                    x_sb[gi] = ([x0] + [xa[:, kt] for kt in range(HH - 1)]
                                + [xb[:, kt] for kt in range(KD - HH)])
                    return (wg0, wg1)

                if s == 0:
                    off0, csize0 = group[0]
                    wg0, wg1 = load_x(0, off0, csize0)
                    w_cache[0] = ([wg0, wg1],
                                  [load_wu(0, 0), load_wu(0, 1)])
                    w_cache[1] = load_w(1)
                    for gi, (off, csize) in enumerate(group):
                        if gi:
                            load_x(gi, off, csize)
                else:
                    w_cache[0] = load_w(0)
                    w_cache[1] = load_w(1)
                    for gi, (off, csize) in enumerate(group):
                        load_x(gi, off, csize)

                for ht in range(HT):
                    if ht not in w_cache:
                        w_cache[ht] = load_w(ht)
                    wg_sb, wu_sb = w_cache.pop(ht)

                    for gi, (off, csize) in enumerate(group):
                        pg = ps.tile([P, csize], f32, tag="pg", bufs=3)
                        pu = ps.tile([P, csize], f32, tag="pu", bufs=3)
                        for kt in range(KD):
                            nc.tensor.matmul(
                                pg,
                                wg_sb[kt // HH][:, kt % HH],
                                x_sb[gi][kt],
                                start=(kt == 0),
                                stop=(kt == KD - 1),
                            )
                        for kt in range(KD):
                            nc.tensor.matmul(
                                pu,
                                wu_sb[kt // HH][:, kt % HH],
                                x_sb[gi][kt],
                                start=(kt == 0),
                                stop=(kt == KD - 1),
                            )
                        sl = op.tile([P, csize], f32, tag="silu")
                        nc.scalar.activation(sl[:], pg, SiLU)
                        nc.vector.tensor_mul(
                            h_sb[:, ht, off : off + csize], sl[:], pu
                        )

                # y = h @ Wd, feature-major [DIM, size]
                for dt in range(DT):
                    wd_sb = wp.tile([P, HT, P], dt_in, tag="wd")
                    nc.sync.dma_start(wd_sb[:], wdt[s][dt])
                    for gi, (off, csize) in enumerate(group):
                        py = ps.tile([P, csize], f32, tag="py")
                        for ht in range(HT):
                            nc.tensor.matmul(
                                py,
                                wd_sb[:, ht],
                                h_sb[:, ht, off : off + csize],
                                start=(ht == 0),
                                stop=(ht == HT - 1),
                            )
                        o_sb = op.tile([P, csize], dt_in, tag="o")
                        # cast AND DMA-trigger both on the Activation
                        # queue: the output DMA fires right after its
                        # cast with no cross-engine semaphore hop
                        # (~1.3us on the final output in the trace)
                        nc.scalar.copy(o_sb[:], py)
                        nc.scalar.dma_start(
                            yt[s][dt, :, off : off + csize], o_sb[:])

    nc.finalize()
    return nc


def _get_kernel(slot_sizes):
    key = tuple(slot_sizes)
    if key not in _KERNEL_CACHE:
        _KERNEL_CACHE[key] = _build_moe_ffn(list(slot_sizes))
    return _KERNEL_CACHE[key]


def _np_bf16():
    import ml_dtypes

    return np.dtype(ml_dtypes.bfloat16)


def _route(xf, W_gate):
    """Replicate reference routing: top-2 by logit, softmax weights.

    float64 logits: the top-k decision boundary gap is >> f32 rounding
    noise, so this matches the f32 jax reference's selection."""
    logits = xf.astype(np.float64) @ W_gate.astype(np.float64)  # [N, E]
    order = np.argsort(-logits, axis=1, kind="stable")[:, :TOPK]  # [N, 2]
    top = np.take_along_axis(logits, order, axis=1)
    top = top - top.max(axis=1, keepdims=True)
    ew = np.exp(top)
    w = (ew / ew.sum(axis=1, keepdims=True)).astype(np.float32)  # [N, 2]
    return order, w


def kernel(x, W_gate, Wg, Wu, Wd):
    from concourse.bass_utils import run_bass_kernel_spmd

    x = np.ascontiguousarray(np.asarray(x, dtype=np.float32))
    W_gate = np.asarray(W_gate, dtype=np.float32)
    Wg = np.asarray(Wg, dtype=np.float32)
    Wu = np.asarray(Wu, dtype=np.float32)
    Wd = np.asarray(Wd, dtype=np.float32)

    B, T, D = x.shape
    xf = x.reshape(-1, D)
    N = xf.shape[0]

    order, w = _route(xf, W_gate)

    ids = []  # per-expert token indices
    wts = []  # per-expert combine weights
    for e in range(E):
        sel = np.nonzero(order == e)
        ids.append(sel[0])
        wts.append(w[sel[0], sel[1]])

    sizes, assign = _plan([len(i) for i in ids])
    nc = _get_kernel(sizes)
    ndt = _np_bf16()

    # transpose + bf16-cast each expert's weights once
    wcache = {}

    def get_w(e):
        if e not in wcache:
            wg_t = np.ascontiguousarray(
                Wg[e].reshape(KD, P, HT, P).transpose(2, 1, 0, 3)
                .astype(ndt, copy=False))
            wu_t = np.ascontiguousarray(
                Wu[e].reshape(KD, P, HT, P).transpose(2, 1, 0, 3)
                .astype(ndt, copy=False))
            wd_t = np.ascontiguousarray(
                Wd[e].reshape(HT, P, DT, P).transpose(2, 1, 0, 3)
                .astype(ndt, copy=False))
            wcache[e] = (wg_t, wu_t, wd_t)
        return wcache[e]

    in_maps = []
    for core in range(E):
        im = {}
        for s, size in enumerate(sizes):
            e, lo, hi = assign[core][s]
            cnt = hi - lo
            xe = np.zeros((size, DIM), dtype=np.float32)
            if cnt:
                xe[:cnt] = xf[ids[e][lo:hi]]
            im[f"xt{s}"] = np.ascontiguousarray(
                xe.T.reshape(KD, P, size).transpose(1, 0, 2)
                .astype(ndt, copy=False))
            wg_t, wu_t, wd_t = get_w(e)
            im[f"wgt{s}"] = wg_t
            im[f"wut{s}"] = wu_t
            im[f"wdt{s}"] = wd_t
        in_maps.append(im)

    res = run_bass_kernel_spmd(nc, in_maps, core_ids=list(range(E)))
    global LAST_RESULTS
    LAST_RESULTS = res

    out = np.zeros((N, D), dtype=np.float32)
    for core in range(E):
        for s, size in enumerate(sizes):
            e, lo, hi = assign[core][s]
            cnt = hi - lo
            if not cnt:
                continue
            y_e = (res.results[core][f"yt{s}"].astype(np.float32)
                   .reshape(DIM, size)[:, :cnt].T)
            out[ids[e][lo:hi]] += wts[e][lo:hi, None] * y_e
    return out.reshape(B, T, D)


# revision 19
# speedup vs baseline: 1.0021x; 1.0010x over previous
"""Expert-parallel MoE (top-2 of 8 experts, SwiGLU FFN) for 8 Trainium2 cores.

Strategy (expert-parallel per the sharding hint, plus load balancing):
  - Host computes the small gate (logits -> top-2 -> softmax) in float64
    numpy, then dispatches tokens to experts.
  - Device work is the 3-matmul SwiGLU FFN in bf16 (1 PE cycle/row, FWL
    hides LDWEIGHTS; rel err ~4e-3 vs the 2e-2 gate). fp8-DoubleRow was
    measured unusable: e4m3 on any one matmul already gives >3.7e-2.
  - Load balancing: per-expert token counts are imbalanced (max 2151 vs
    mean 2048).  Instead of padding every core to the max count, each
    core runs THREE fixed-size slots (a, b, c); the biggest expert's
    tokens span two a-slots (on different cores), the smallest expert
    donates capacity by using two b-slots, and everyone gets one
    c-slot.  This brings per-core capacity S = a+b+c down to
    max(d2, ceil((d1+d8)/2)) ~ 2068 instead of 2152.  Each slot streams
    its own expert's weights (3 x 17.3MB bf16 per core, well under the
    HBM roofline vs ~455us of compute).
  - The device kernel works in "feature-major" layout (features on
    partitions, tokens on the free axis) so h = silu(x@Wg)*(x@Wu)
    feeds the down-projection without any transpose.
  - Host applies routing weights and scatter-adds per-slot outputs
    back into the full [B,T,D] output.
"""

import numpy as np

DIM = 1024
HID = 2816
E = 8
TOPK = 2
P = 128
KD = DIM // P   # 8 k-subtiles (contraction of x@W)
HT = HID // P   # 22 h-subtiles
DT = DIM // P   # 8 d-subtiles (output features)

CHUNK_MAX = 512         # PSUM bank limit (512 fp32 accumulators)

_KERNEL_CACHE = {}
LAST_RESULTS = None  # BassKernelResults of the most recent run (for profiling)


def _ceil_even(v):
    return ((int(v) + 1) // 2) * 2


def _split_chunks(size):
    """Split a slot into even chunks <= CHUNK_MAX (one PSUM bank each),
    near-equal so every chunk's matmuls stay long enough to hide
    LDWEIGHTS."""
    n = -(-size // CHUNK_MAX)
    sizes = []
    rem = size
    for i in range(n, 0, -1):
        s = rem if i == 1 else min(CHUNK_MAX, ((rem // i + 1) // 2) * 2)
        sizes.append(s)
        rem -= s
    assert rem == 0 and all(0 < s <= CHUNK_MAX and s % 2 == 0 for s in sizes)
    chunks = []
    off = 0
    for s in sizes:
        chunks.append((off, s))
        off += s
    return chunks


def _plan(counts):
    """Choose per-core slot sizes + (expert, lo, hi) piece assignment.

    Slot types (a, b, c), 8 copies of each (one per core):
      biggest expert  -> {a, a, c}   (its surplus spans two cores' a-slots)
      smallest expert -> {b, b, c}   (donates capacity: b < a)
      middle experts  -> {a, b, c}
    Coverage: 2a+c >= d_max, 2b+c >= d_min, a+b+c >= d_2nd, so
    S = a+b+c = max(d_2nd, ceil((d_max+d_min)/2)) is achievable.
    Returns (sizes, assign) with sizes desc-sorted and
    assign[core] = [(expert, lo, hi), ...] parallel to sizes.
    """
    cnts = [int(x) for x in counts]
    order = sorted(range(E), key=lambda e: -cnts[e])
    d = [cnts[e] for e in order]
    d0, d1, dmin = d[0], d[1], d[-1]

    smin = _ceil_even(max(d1, (d0 + dmin + 1) // 2, 6))
    best = None
    for c in range(384, 1153, 32):
        if c >= smin - 4:
            break
        S = smin
        plan_ab = None
        while S <= _ceil_even(d0):
            a = _ceil_even(-(-max(d0 - c, 2) // 2))
            b = S - a - c
            if b >= 2 and 2 * b + c >= dmin:
                plan_ab = (a, b)
                break
            S += 2
        if plan_ab is None:
            continue
        a, b = plan_ab
        minchunk = min(
            min(cs for _, cs in _split_chunks(s)) for s in (a, b, c)
        )
        key = (S, -minchunk)
        if best is None or key < best[0]:
            best = (key, (a, b, c))

    if best is None:
        # fallback: uniform single slot per core (always correct)
        C = _ceil_even(max(max(cnts), 256))
        sizes = [C]
        assign = [[(order[i], 0, cnts[order[i]])] for i in range(E)]
        return sizes, assign

    a, b, c = best[1]

    def pieces(n, caps):
        out = []
        lo = 0
        for cap in caps:
            hi = min(n, lo + cap)
            out.append((lo, hi))
            lo = hi
        assert lo == n
        return out

    a_pieces, b_pieces, c_pieces = [], [], []
    for rank, e in enumerate(order):
        n = cnts[e]
        if rank == 0:
            pa1, pa2, pc = pieces(n, [a, a, c])
            a_pieces += [(e,) + pa1, (e,) + pa2]
            c_pieces.append((e,) + pc)
        elif rank == E - 1:
            pb1, pb2, pc = pieces(n, [b, b, c])
            b_pieces += [(e,) + pb1, (e,) + pb2]
            c_pieces.append((e,) + pc)
        else:
            pa, pb, pc = pieces(n, [a, b, c])
            a_pieces.append((e,) + pa)
            b_pieces.append((e,) + pb)
            c_pieces.append((e,) + pc)
    assert len(a_pieces) == E and len(b_pieces) == E and len(c_pieces) == E

    typed = sorted(
        [(a, a_pieces), (b, b_pieces), (c, c_pieces)], key=lambda t: -t[0]
    )
    sizes = [t[0] for t in typed]
    assign = [[typed[s][1][i] for s in range(len(typed))] for i in range(E)]
    return sizes, assign


def _build_moe_ffn(slot_sizes):
    """Per-core Bass program: for each slot, y^T = SwiGLU FFN of x^T with
    that slot's own expert weights; feature-major, bf16 matmuls."""
    import concourse.bass as bass  # noqa: F401
    import concourse.mybir as mybir
    from concourse import bacc, tile

    f32 = mybir.dt.float32
    dt_in = mybir.dt.bfloat16
    SiLU = mybir.ActivationFunctionType.Silu

    nc = bacc.Bacc("TRN2", target_bir_lowering=False, debug=False)

    xt, wgt, wut, wdt, yt = [], [], [], [], []
    for s, size in enumerate(slot_sizes):
        xt.append(nc.dram_tensor(f"xt{s}", [P, KD, size], dt_in,
                                 kind="ExternalInput"))
        wgt.append(nc.dram_tensor(f"wgt{s}", [HT, P, KD, P], dt_in,
                                  kind="ExternalInput"))
        wut.append(nc.dram_tensor(f"wut{s}", [HT, P, KD, P], dt_in,
                                  kind="ExternalInput"))
        wdt.append(nc.dram_tensor(f"wdt{s}", [DT, P, HT, P], dt_in,
                                  kind="ExternalInput"))
        yt.append(nc.dram_tensor(f"yt{s}", [DT, P, size], dt_in,
                                 kind="ExternalOutput"))

    with tile.TileContext(nc) as tc:
        with (
            tc.tile_pool(name="xp", bufs=1) as xp,
            tc.tile_pool(name="wp", bufs=3) as wp,
            tc.tile_pool(name="hp", bufs=2) as hp,
            tc.tile_pool(name="op", bufs=3) as op,
            tc.tile_pool(name="ps", bufs=2, space="PSUM") as ps,
        ):
            HH = KD // 2  # wg/wu tiles split in halves for earlier start

            # Warm-up: ~10us of dummy matmuls on memset tiles keep the
            # PE HAM clock-gate busy during the opening DMA fill, so the
            # first real matmuls run at 2.4GHz instead of 1.2.
            wdum = op.tile([P, P], mybir.dt.bfloat16, tag="wdum", bufs=1)
            nc.vector.memset(wdum[:], 0.0)
            xdum = op.tile([P, CHUNK_MAX], mybir.dt.bfloat16, tag="xdum",
                           bufs=1)
            nc.vector.memset(xdum[:], 0.0)
            for _ in range(12):
                pwarm = ps.tile([P, CHUNK_MAX], mybir.dt.float32,
                                tag="pg", bufs=3, name="pwarm")
                nc.tensor.matmul(pwarm, wdum[:], xdum[:],
                                 start=True, stop=True)

            for s, size in enumerate(slot_sizes):
                group = _split_chunks(size)

                # h = silu(x @ Wg) * (x @ Wu), feature-major [HID, size]
                h_sb = hp.tile([P, HT, size], dt_in, tag="h")

                w_cache = {}

                def load_wg(ht, hh, s=s):
                    w1 = wp.tile([P, HH, P], dt_in, tag=f"wg{hh}",
                                 name=f"wg{hh}")
                    nc.sync.dma_start(
                        w1[:], wgt[s][ht, :, hh * HH : (hh + 1) * HH])
                    return w1

                def load_wu(ht, hh, s=s):
                    w2 = wp.tile([P, HH, P], dt_in, tag=f"wu{hh}",
                                 name=f"wu{hh}")
                    nc.sync.dma_start(
                        w2[:], wut[s][ht, :, hh * HH : (hh + 1) * HH])
                    return w2

                def load_w(ht):
                    # both wg halves before wu: the opening matmuls of
                    # each h-tile consume wg only
                    return ([load_wg(ht, 0), load_wg(ht, 1)],
                            [load_wu(ht, 0), load_wu(ht, 1)])

                # Opening order interleaves the first chunk's x k-slices
                # with ht=0's weight tiles so the first matmul waits on
                # ~240KB and each successive k-group lands just in time.
                x_sb = [None] * len(group)

                def load_x(gi, off, csize, s=s):
                    x0 = xp.tile([P, csize], dt_in, tag=f"x{gi}k0",
                                 bufs=2 if gi == 0 else 1, name=f"x{gi}k0")
                    nc.sync.dma_start(x0[:], xt[s][:, 0, off : off + csize])
                    wg0 = load_wg(0, 0) if gi == 0 and s == 0 else None
                    xa = xp.tile([P, HH - 1, csize], dt_in, tag=f"x{gi}a",
                                 bufs=2 if gi == 0 else 1, name=f"x{gi}a")
                    nc.sync.dma_start(xa[:], xt[s][:, 1:HH, off : off + csize])
                    wg1 = load_wg(0, 1) if gi == 0 and s == 0 else None
                    xb = xp.tile([P, KD - HH, csize], dt_in, tag=f"x{gi}b",
                                 bufs=2 if gi == 0 else 1, name=f"x{gi}b")
                    nc.sync.dma_start(xb[:], xt[s][:, HH:, off : off + csize])
                    x_sb[gi] = ([x0] + [xa[:, kt] for kt in range(HH - 1)]
                                + [xb[:, kt] for kt in range(KD - HH)])
                    return (wg0, wg1)

                if s == 0:
                    off0, csize0 = group[0]
                    wg0, wg1 = load_x(0, off0, csize0)
                    w_cache[0] = ([wg0, wg1],
                                  [load_wu(0, 0), load_wu(0, 1)])
                    w_cache[1] = load_w(1)
                    for gi, (off, csize) in enumerate(group):
                        if gi:
                            load_x(gi, off, csize)
                else:
                    w_cache[0] = load_w(0)
                    w_cache[1] = load_w(1)
                    for gi, (off, csize) in enumerate(group):
                        load_x(gi, off, csize)

                for ht in range(HT):
                    if ht not in w_cache:
                        w_cache[ht] = load_w(ht)
                    wg_sb, wu_sb = w_cache.pop(ht)

                    for gi, (off, csize) in enumerate(group):
                        pg = ps.tile([P, csize], f32, tag="pg", bufs=3)
                        pu = ps.tile([P, csize], f32, tag="pu", bufs=3)
                        for kt in range(KD):
                            nc.tensor.matmul(
                                pg,
                                wg_sb[kt // HH][:, kt % HH],
                                x_sb[gi][kt],
                                start=(kt == 0),
                                stop=(kt == KD - 1),
                            )
                        for kt in range(KD):
                            nc.tensor.matmul(
                                pu,
                                wu_sb[kt // HH][:, kt % HH],
                                x_sb[gi][kt],
                                start=(kt == 0),
                                stop=(kt == KD - 1),
                            )
                        sl = op.tile([P, csize], f32, tag="silu")
                        nc.scalar.activation(sl[:], pg, SiLU)
                        nc.vector.tensor_mul(
                            h_sb[:, ht, off : off + csize], sl[:], pu
                        )

                # y = h @ Wd, feature-major [DIM, size]
                for dt in range(DT):
                    wd_sb = wp.tile([P, HT, P], dt_in, tag="wd")
                    nc.sync.dma_start(wd_sb[:], wdt[s][dt])
                    for gi, (off, csize) in enumerate(group):
                        py = ps.tile([P, csize], f32, tag="py")
                        for ht in range(HT):
                            nc.tensor.matmul(
                                py,
                                wd_sb[:, ht],
                                h_sb[:, ht, off : off + csize],
                                start=(ht == 0),
                                stop=(ht == HT - 1),
                            )
                        o_sb = op.tile([P, csize], dt_in, tag="o")
                        nc.vector.tensor_copy(o_sb[:], py)
                        nc.sync.dma_start(
                            yt[s][dt, :, off : off + csize], o_sb[:])

    nc.finalize()
    return nc


def _get_kernel(slot_sizes):
    key = tuple(slot_sizes)
    if key not in _KERNEL_CACHE:
        _KERNEL_CACHE[key] = _build_moe_ffn(list(slot_sizes))
    return _KERNEL_CACHE[key]


def _np_bf16():
    import ml_dtypes

    return np.dtype(ml_dtypes.bfloat16)


def _route(xf, W_gate):
    """Replicate reference routing: top-2 by logit, softmax weights.

    float64 logits: the top-k decision boundary gap is >> f32 rounding
    noise, so this matches the f32 jax reference's selection."""
    logits = xf.astype(np.float64) @ W_gate.astype(np.float64)  # [N, E]
    order = np.argsort(-logits, axis=1, kind="stable")[:, :TOPK]  # [N, 2]
    top = np.take_along_axis(logits, order, axis=1)
    top = top - top.max(axis=1, keepdims=True)
    ew = np.exp(top)
    w = (ew / ew.sum(axis=1, keepdims=True)).astype(np.float32)  # [N, 2]
    return order, w


def kernel(x, W_gate, Wg, Wu, Wd):
    from concourse.bass_utils import run_bass_kernel_spmd

    x = np.ascontiguousarray(np.asarray(x, dtype=np.float32))
    W_gate = np.asarray(W_gate, dtype=np.float32)
    Wg = np.asarray(Wg, dtype=np.float32)
    Wu = np.asarray(Wu, dtype=np.float32)
    Wd = np.asarray(Wd, dtype=np.float32)

    B, T, D = x.shape
    xf = x.reshape(-1, D)
    N = xf.shape[0]

    order, w = _route(xf, W_gate)

    ids = []  # per-expert token indices
    wts = []  # per-expert combine weights
    for e in range(E):
        sel = np.nonzero(order == e)
        ids.append(sel[0])
        wts.append(w[sel[0], sel[1]])

    sizes, assign = _plan([len(i) for i in ids])
    nc = _get_kernel(sizes)
    ndt = _np_bf16()

    # transpose + bf16-cast each expert's weights once
    wcache = {}

    def get_w(e):
        if e not in wcache:
            wg_t = np.ascontiguousarray(
                Wg[e].reshape(KD, P, HT, P).transpose(2, 1, 0, 3)
                .astype(ndt, copy=False))
            wu_t = np.ascontiguousarray(
                Wu[e].reshape(KD, P, HT, P).transpose(2, 1, 0, 3)
                .astype(ndt, copy=False))
            wd_t = np.ascontiguousarray(
                Wd[e].reshape(HT, P, DT, P).transpose(2, 1, 0, 3)
                .astype(ndt, copy=False))
            wcache[e] = (wg_t, wu_t, wd_t)
        return wcache[e]

    in_maps = []
    for core in range(E):
        im = {}
        for s, size in enumerate(sizes):
            e, lo, hi = assign[core][s]
            cnt = hi - lo
            xe = np.zeros((size, DIM), dtype=np.float32)
            if cnt:
                xe[:cnt] = xf[ids[e][lo:hi]]
            im[f"xt{s}"] = np.ascontiguousarray(
                xe.T.reshape(KD, P, size).transpose(1, 0, 2)
                .astype(ndt, copy=False))
            wg_t, wu_t, wd_t = get_w(e)
            im[f"wgt{s}"] = wg_t
            im[f"wut{s}"] = wu_t
            im[f"wdt{s}"] = wd_t
        in_maps.append(im)

    res = run_bass_kernel_spmd(nc, in_maps, core_ids=list(range(E)))
    global LAST_RESULTS
    LAST_RESULTS = res

    out = np.zeros((N, D), dtype=np.float32)
    for core in range(E):
        for s, size in enumerate(sizes):
            e, lo, hi = assign[core][s]
            cnt = hi - lo
            if not cnt:
                continue
            y_e = (res.results[core][f"yt{s}"].astype(np.float32)
                   .reshape(DIM, size)[:, :cnt].T)
            out[ids[e][lo:hi]] += wts[e][lo:hi, None] * y_e
    return out.reshape(B, T, D)
